# revision 2
# baseline (speedup 1.0000x reference)
"""Trainium2 Bass kernel for nn_LogisticRegressionModel (polynomial-feature logistic regression).

Math: the reference computes sigmoid(poly_features(x) @ W.T + b) where poly_features
are all monomials of x (dim 16) up to degree 4, each degree soft-weighted by
w_d = sigmoid(10*(M - d + 0.5)), M = sigmoid(M_raw)*3 + 1.

Every monomial of degree <= 4 over x embeds as a degree-4 monomial over x1 = [x, 1]
(pad with the constant slot, index 16). Folding W, b, M_raw into a symmetrized
coefficient tensor S4 [289, 289] (built on host, O(P) work), the model becomes
logit_i = (x1 (x) x1)^T S4 (x1 (x) x1). The outer product is symmetric, so it is
further folded onto the 153 unordered pairs of 17 symbols using a mod-17 "wrap"
enumeration p=(d,j) <-> {j, (j+d)%17}, d=0..8: S153 = B^T S4 B. The wrap pairs
have regular strides against a doubled x1 buffer, so one DVE tensor_tensor with
broadcast APs builds XXs[128,153] per batch tile. Then PE transposes XXs
(2 chunks), 2 accumulating matmuls against resident S153 give Y = XXs @ S153,
and a fused scalar_tensor_tensor computes q = rowsum(XXs * Y); sigmoid; store.

Sharding: pure data-parallel over the batch, 4096 rows per core x 8 cores.
"""
import sys
import numpy as np
from itertools import combinations_with_replacement, permutations

sys.path.insert(0, "/opt/trn_rl_repo")

import concourse.bass as bass
import concourse.bacc as bacc
import concourse.tile as tile
from concourse import mybir, masks
from concourse import bass_utils

BATCH = 32768
D = 16
DA = 17            # features + constant slot
ND = 9             # wrap distances 0..8
PD = ND * DA       # 153 unordered pairs
MAX_DEGREE = 4
N_CORES = 8
B_CORE = BATCH // N_CORES   # 4096
N_TILES = B_CORE // 128     # 32
KCH = [128, PD - 128]       # 153 split across partition chunks
P_FULL = 1 + sum(
    len(list(combinations_with_replacement(range(D), d))) for d in range(1, MAX_DEGREE + 1)
)


def _build_s153(W, b, M_raw):
    """Fold W, b and the soft degree weights into the symmetric quartic
    coefficient matrix over the 153 wrap-encoded unordered pairs."""
    W = np.asarray(W, np.float64)
    bval = float(np.asarray(b).reshape(-1)[0])
    M = 1.0 / (1.0 + np.exp(-float(np.asarray(M_raw)))) * (MAX_DEGREE - 1) + 1.0
    coef = {(16, 16, 16, 16): float(W[0, 0]) + bval}
    col = 1
    for d in range(1, MAX_DEGREE + 1):
        w_d = 1.0 / (1.0 + np.exp(-10.0 * (M - d + 0.5)))
        for t in combinations_with_replacement(range(D), d):
            tup = tuple(sorted(t + (16,) * (4 - d)))
            coef[tup] = float(W[0, col]) * w_d
            col += 1
    assert col == P_FULL
    S4 = np.zeros((DA * DA, DA * DA), np.float64)
    for tup, c in coef.items():
        perms = set(permutations(tup))
        v = c / len(perms)
        for (a, b2, c2, d2) in perms:
            S4[a * DA + b2, c2 * DA + d2] += v
    # fold ordered 289-space onto wrap-encoded 153-space
    lookup = {}
    for p, (a, c) in enumerate((j, (j + dd) % DA) for dd in range(ND) for j in range(DA)):
        lookup[(a, c)] = p
        lookup[(c, a)] = p
    B = np.zeros((DA * DA, PD))
    for j in range(DA):
        for k in range(DA):
            B[j * DA + k, lookup[(j, k)]] = 1.0
    return (B.T @ S4 @ B).astype(np.float32)


def _build_nc():
    nc = bacc.Bacc("TRN2", target_bir_lowering=False, debug=False, enable_asserts=False)
    # host pre-packs x1 as [128, N_TILES*34]: partition p, tile t holds x1[t*128+p] twice
    x_d = nc.dram_tensor("x", [128, N_TILES * 2 * DA], mybir.dt.float32, kind="ExternalInput").ap()
    s_d = nc.dram_tensor("s4", [PD, PD], mybir.dt.float32, kind="ExternalInput").ap()
    out_d = nc.dram_tensor("out", [B_CORE, 1], mybir.dt.float32, kind="ExternalOutput").ap()

    f32 = mybir.dt.float32
    with tile.TileContext(nc) as tc:
        with (
            tc.tile_pool(name="const", bufs=1) as const_pool,
            tc.tile_pool(name="xx", bufs=5) as xx_pool,
            tc.tile_pool(name="xxt", bufs=4) as xxt_pool,
            tc.tile_pool(name="prod", bufs=3) as prod_pool,
            tc.tile_pool(name="tr_ps", bufs=5, space="PSUM") as trps_pool,
            tc.tile_pool(name="y_ps", bufs=2, space="PSUM") as yps_pool,
            tc.tile_pool(name="o_ps", bufs=1, space="PSUM") as ops_pool,
        ):
            ident = const_pool.tile([128, 128], f32)
            masks.make_identity(nc, ident[:])
            # S153 chunks: chunk c lives at [:KCH[c], c*153:(c+1)*153]
            s_sb = const_pool.tile([128, 2 * PD], f32)
            for c in range(2):
                nc.sync.dma_start(
                    out=s_sb[: KCH[c], c * PD : (c + 1) * PD],
                    in_=s_d[c * 128 : c * 128 + KCH[c], :],
                )
            qall = const_pool.tile([128, N_TILES], f32)
            oall = const_pool.tile([128, N_TILES], f32)
            xall = const_pool.tile([128, N_TILES * 2 * DA], f32)
            nc.sync.dma_start(out=xall[:], in_=x_d[:])
            # collapse the prologue's many DMA-queue semaphores into one edge
            tc.strict_bb_all_engine_barrier()

            for t in range(N_TILES):
                xc = xall[:, t * 2 * DA : (t + 1) * 2 * DA]

                # XXs[p, d*17+j] = x1[p,j] * x1[p,(j+d)%17] — one DVE op
                xx = xx_pool.tile([128, PD], f32)
                in1 = bass.AP(xc.tensor, xc.offset, [list(xc.ap[0]), [1, ND], [1, DA]])
                nc.vector.tensor_tensor(
                    out=xx[:].rearrange("p (d j) -> p d j", d=ND),
                    in0=xc[:, :DA].unsqueeze(1).broadcast_to([128, ND, DA]),
                    in1=in1,
                    op=mybir.AluOpType.mult,
                )

                # Transpose XXs -> chunks [KCH[c], 128] at cols c*128
                xxt = xxt_pool.tile([128, 2 * 128], f32)
                for c in range(2):
                    trp = trps_pool.tile([128, 128], f32)
                    nc.tensor.transpose(
                        out=trp[: KCH[c], :],
                        in_=xx[:, c * 128 : c * 128 + KCH[c]],
                        identity=ident[:],
                    )
                    nc.scalar.copy(
                        out=xxt[: KCH[c], c * 128 : c * 128 + 128],
                        in_=trp[: KCH[c], :],
                    )

                # Y = XXs @ S153  [128, 153] accumulated over 2 K-chunks
                y_ps = yps_pool.tile([128, PD], f32)
                for c in range(2):
                    nc.tensor.matmul(
                        out=y_ps[:],
                        lhsT=xxt[: KCH[c], c * 128 : c * 128 + 128],
                        rhs=s_sb[: KCH[c], c * PD : (c + 1) * PD],
                        start=(c == 0),
                        stop=(c == 1),
                    )

                # q = rowsum(XXs * Y) — fused multiply + accumulate on DVE
                prod = prod_pool.tile([128, PD], f32)
                nc.vector.scalar_tensor_tensor(
                    out=prod[:],
                    in0=xx[:],
                    scalar=1.0,
                    in1=y_ps[:],
                    op0=mybir.AluOpType.bypass,
                    op1=mybir.AluOpType.mult,
                    accum_out=qall[:, t : t + 1],
                )

            # sigmoid over all 32 tile-columns at once
            nc.scalar.activation(oall[:], qall[:], mybir.ActivationFunctionType.Sigmoid)
            # transpose [128, 32] -> [32, 128] so the DRAM store is contiguous
            o_ps = ops_pool.tile([N_TILES, 128], f32)
            nc.tensor.transpose(out=o_ps[:], in_=oall[:], identity=ident[:])
            o_sb = const_pool.tile([N_TILES, 128], f32)
            nc.vector.tensor_copy(out=o_sb[:], in_=o_ps[:])
            nc.sync.dma_start(
                out=out_d.rearrange("(t p) one -> t (p one)", p=128),
                in_=o_sb[:],
            )
    nc.compile()
    return nc


_NC_CACHE = None


def _pack_inputs(x, W, b, M_raw):
    x = np.asarray(x, np.float32)
    x1 = np.concatenate([x, np.ones((x.shape[0], 1), np.float32)], axis=1)
    # pack per core: [N_TILES, 128, 17] -> [128, N_TILES, 2*17] (doubled for wrap reads)
    xr = x1.reshape(N_CORES, N_TILES, 128, DA)
    xp = np.concatenate([xr, xr], axis=3).transpose(0, 2, 1, 3)  # [C, 128, T, 34]
    xp = np.ascontiguousarray(xp.reshape(N_CORES, 128, N_TILES * 2 * DA))
    S = _build_s153(W, b, M_raw)
    return [{"x": xp[i], "s4": S} for i in range(N_CORES)]


def kernel(x, W, b, M_raw):
    global _NC_CACHE
    in_maps = _pack_inputs(x, W, b, M_raw)
    if _NC_CACHE is None:
        _NC_CACHE = _build_nc()
    nc = _NC_CACHE
    res = bass_utils.run_bass_kernel_spmd(nc, in_maps, core_ids=list(range(N_CORES)))
    out = np.concatenate([res.results[i]["out"] for i in range(N_CORES)], axis=0)
    return out.reshape(BATCH, 1).astype(np.float32)


if __name__ == "__main__":
    x = np.random.randn(BATCH, D).astype(np.float32)
    W = (np.random.randn(1, P_FULL) * 0.02).astype(np.float32)
    b = np.zeros((1,), np.float32)
    M_raw = np.zeros((), np.float32)
    out = kernel(x, W, b, M_raw)
    print("out shape:", out.shape, out.dtype, out[:4, 0])



# revision 6
# speedup vs baseline: 1.6171x; 1.6171x over previous
"""Trainium2 Bass kernel for nn_LogisticRegressionModel (polynomial-feature logistic regression).

Math: the reference computes sigmoid(poly_features(x) @ W.T + b) where poly_features
are all monomials of x (dim 16) up to degree 4, each degree soft-weighted by
w_d = sigmoid(10*(M - d + 0.5)), M = sigmoid(M_raw)*3 + 1.

Every monomial of degree <= 4 over x embeds as a degree-4 monomial over x1 = [x, 1]
(pad with the constant slot, index 16). Folding W, b, M_raw into a symmetrized
coefficient tensor S4 [289, 289] (built on host, O(P) work), the model becomes
logit_i = (x1 (x) x1)^T S4 (x1 (x) x1). The outer product is symmetric, so it is
further folded onto the 153 unordered pairs of 17 symbols using a mod-17 "wrap"
enumeration p=(d,j) <-> {j, (j+d)%17}, d=0..8: S153 = B^T S4 B. The wrap pairs
have regular strides against a doubled x1 buffer, so one DVE tensor_tensor with
broadcast APs builds XXs[128, 2*153] per pair of batch tiles. Then PE transposes
XXs (2 chunks), 2 accumulating matmuls against resident S153 give Y = XXs @ S153,
and a fused scalar_tensor_tensor computes q = rowsum(XXs * Y); sigmoid; store.

The entire multiply pipeline runs in bf16 (inputs, S, transposes, matmuls) with
fp32 PSUM accumulation: q is in [-2.5, 2.5] and sigmoid never saturates, so
bf16 rounding keeps max rel err ~3e-3 (validated numerically). bf16 makes the
PE run at 1 cycle/row (vs 4 for fp32's LOW/HIGH double pass) and halves DMA.

Sharding: pure data-parallel over the batch, 4096 rows per core x 8 cores.
"""
import sys
import numpy as np
import ml_dtypes
from itertools import combinations_with_replacement, permutations

sys.path.insert(0, "/opt/trn_rl_repo")

import concourse.bass as bass
import concourse.bacc as bacc
import concourse.tile as tile
from concourse import mybir, masks
from concourse import bass_utils

BATCH = 32768
D = 16
DA = 17            # features + constant slot
ND = 9             # wrap distances 0..8
PD = ND * DA       # 153 unordered pairs
MAX_DEGREE = 4
N_CORES = 8
B_CORE = BATCH // N_CORES   # 4096
N_TILES = B_CORE // 128     # 32
KCH = [128, PD - 128]       # 153 split across partition chunks
NXCH = 8                    # x DMA chunks (4 tiles each)
P_FULL = 1 + sum(
    len(list(combinations_with_replacement(range(D), d))) for d in range(1, MAX_DEGREE + 1)
)
BF16 = ml_dtypes.bfloat16


def _build_s153(W, b, M_raw):
    """Fold W, b and the soft degree weights into the symmetric quartic
    coefficient matrix over the 153 wrap-encoded unordered pairs."""
    W = np.asarray(W, np.float64)
    bval = float(np.asarray(b).reshape(-1)[0])
    M = 1.0 / (1.0 + np.exp(-float(np.asarray(M_raw)))) * (MAX_DEGREE - 1) + 1.0
    coef = {(16, 16, 16, 16): float(W[0, 0]) + bval}
    col = 1
    for d in range(1, MAX_DEGREE + 1):
        w_d = 1.0 / (1.0 + np.exp(-10.0 * (M - d + 0.5)))
        for t in combinations_with_replacement(range(D), d):
            tup = tuple(sorted(t + (16,) * (4 - d)))
            coef[tup] = float(W[0, col]) * w_d
            col += 1
    assert col == P_FULL
    S4 = np.zeros((DA * DA, DA * DA), np.float64)
    for tup, c in coef.items():
        perms = set(permutations(tup))
        v = c / len(perms)
        for (a, b2, c2, d2) in perms:
            S4[a * DA + b2, c2 * DA + d2] += v
    # fold ordered 289-space onto wrap-encoded 153-space
    lookup = {}
    for p, (a, c) in enumerate((j, (j + dd) % DA) for dd in range(ND) for j in range(DA)):
        lookup[(a, c)] = p
        lookup[(c, a)] = p
    B = np.zeros((DA * DA, PD))
    for j in range(DA):
        for k in range(DA):
            B[j * DA + k, lookup[(j, k)]] = 1.0
    return (B.T @ S4 @ B).astype(np.float32)


def _build_nc():
    nc = bacc.Bacc("TRN2", target_bir_lowering=False, debug=False, enable_asserts=False)
    # host pre-packs x1 as [128, N_TILES*34] bf16: partition p, tile t holds x1[t*128+p] twice
    x_d = nc.dram_tensor("x", [128, N_TILES * 2 * DA], mybir.dt.bfloat16, kind="ExternalInput").ap()
    s_d = nc.dram_tensor("s4", [PD, PD], mybir.dt.bfloat16, kind="ExternalInput").ap()
    out_d = nc.dram_tensor("out", [B_CORE, 1], mybir.dt.float32, kind="ExternalOutput").ap()

    f32 = mybir.dt.float32
    bf16 = mybir.dt.bfloat16
    T_PER_CH = N_TILES // NXCH
    with tile.TileContext(nc) as tc:
        with (
            tc.tile_pool(name="const", bufs=1) as const_pool,
            tc.tile_pool(name="xx", bufs=3) as xx_pool,
            tc.tile_pool(name="xxt", bufs=4) as xxt_pool,
            tc.tile_pool(name="prod", bufs=2) as prod_pool,
            tc.tile_pool(name="tr_ps", bufs=3, space="PSUM") as trps_pool,
            tc.tile_pool(name="y_ps", bufs=2, space="PSUM") as yps_pool,
            tc.tile_pool(name="o_ps", bufs=1, space="PSUM") as ops_pool,
        ):
            ident_b = const_pool.tile([128, 128], bf16)
            masks.make_identity(nc, ident_b[:])
            ident_f = const_pool.tile([128, 128], f32)
            masks.make_identity(nc, ident_f[:])
            # S153 chunks: chunk c lives at [:KCH[c], c*153:(c+1)*153]
            s_sb = const_pool.tile([128, 2 * PD], bf16)
            for c in range(2):
                nc.sync.dma_start(
                    out=s_sb[: KCH[c], c * PD : (c + 1) * PD],
                    in_=s_d[c * 128 : c * 128 + KCH[c], :],
                )
            qall = const_pool.tile([128, N_TILES], f32)
            oall = const_pool.tile([128, N_TILES], f32)
            # x arrives in NXCH chunks so compute can start after the first one
            xch = []
            for k in range(NXCH):
                xc = const_pool.tile([128, T_PER_CH * 2 * DA], bf16)
                nc.sync.dma_start(
                    out=xc[:],
                    in_=x_d[:, k * T_PER_CH * 2 * DA : (k + 1) * T_PER_CH * 2 * DA],
                )
                xch.append(xc)

            for t in range(N_TILES):
                g, gi = t // 2, t % 2
                if gi == 0:
                    # XXs[p, t2, d*17+j] = x1[p,j] * x1[p,(j+d)%17] for 2 tiles — one DVE op
                    xx2 = xx_pool.tile([128, 2 * PD], bf16)
                    ch = xch[t // T_PER_CH]
                    xc2 = ch[:, (t % T_PER_CH) * 2 * DA : (t % T_PER_CH + 2) * 2 * DA]
                    part = list(xc2.ap[0])
                    in0 = bass.AP(
                        xc2.tensor, xc2.offset, [part, [2 * DA, 2], [0, ND], [1, DA]]
                    )
                    in1 = bass.AP(
                        xc2.tensor, xc2.offset, [part, [2 * DA, 2], [1, ND], [1, DA]]
                    )
                    nc.gpsimd.tensor_tensor(
                        out=xx2[:].rearrange("p (t2 d j) -> p t2 d j", t2=2, d=ND),
                        in0=in0,
                        in1=in1,
                        op=mybir.AluOpType.mult,
                    )
                xx = xx2[:, gi * PD : (gi + 1) * PD]

                # Transpose XXs -> chunks [KCH[c], 128] at cols c*128 of one PSUM tile
                trp = trps_pool.tile([128, 256], bf16)
                for c in range(2):
                    nc.tensor.transpose(
                        out=trp[: KCH[c], c * 128 : c * 128 + 128],
                        in_=xx[:, c * 128 : c * 128 + KCH[c]],
                        identity=ident_b[:],
                    )
                # single PSUM->SBUF copy for both chunks; alternate scalar/gpsimd
                xxt = xxt_pool.tile([128, 256], bf16)
                if t % 2 == 0:
                    nc.scalar.copy(out=xxt[:], in_=trp[:])
                else:
                    nc.vector.tensor_copy(out=xxt[:], in_=trp[:])

                # Y = XXs @ S153  [128, 153] accumulated over 2 K-chunks
                if gi == 0:
                    y2 = yps_pool.tile([128, 2 * PD], f32)
                y_sl = y2[:, gi * PD : (gi + 1) * PD]
                for c in range(2):
                    nc.tensor.matmul(
                        out=y_sl,
                        lhsT=xxt[: KCH[c], c * 128 : c * 128 + 128],
                        rhs=s_sb[: KCH[c], c * PD : (c + 1) * PD],
                        start=(c == 0),
                        stop=(c == 1),
                    )

                # q = rowsum(XXs * Y) — fused multiply + accumulate on DVE
                prod = prod_pool.tile([128, PD], bf16)
                nc.vector.scalar_tensor_tensor(
                    out=prod[:],
                    in0=xx,
                    scalar=1.0,
                    in1=y_sl,
                    op0=mybir.AluOpType.bypass,
                    op1=mybir.AluOpType.mult,
                    accum_out=qall[:, t : t + 1],
                )

            # sigmoid over all 32 tile-columns at once
            nc.scalar.activation(oall[:], qall[:], mybir.ActivationFunctionType.Sigmoid)
            # transpose [128, 32] -> [32, 128] so the DRAM store is contiguous
            o_ps = ops_pool.tile([N_TILES, 128], f32)
            nc.tensor.transpose(out=o_ps[:], in_=oall[:], identity=ident_f[:])
            o_sb = const_pool.tile([N_TILES, 128], f32)
            nc.vector.tensor_copy(out=o_sb[:], in_=o_ps[:])
            nc.sync.dma_start(
                out=out_d.rearrange("(t p) one -> t (p one)", p=128),
                in_=o_sb[:],
            )
    nc.compile()
    return nc


_NC_CACHE = None


def _pack_inputs(x, W, b, M_raw):
    x = np.asarray(x, np.float32)
    x1 = np.concatenate([x, np.ones((x.shape[0], 1), np.float32)], axis=1)
    # pack per core: [N_TILES, 128, 17] -> [128, N_TILES, 2*17] (doubled for wrap reads)
    xr = x1.reshape(N_CORES, N_TILES, 128, DA)
    xp = np.concatenate([xr, xr], axis=3).transpose(0, 2, 1, 3)  # [C, 128, T, 34]
    xp = np.ascontiguousarray(
        xp.reshape(N_CORES, 128, N_TILES * 2 * DA).astype(BF16)
    )
    S = _build_s153(W, b, M_raw).astype(BF16)
    return [{"x": xp[i], "s4": S} for i in range(N_CORES)]


def kernel(x, W, b, M_raw):
    global _NC_CACHE
    in_maps = _pack_inputs(x, W, b, M_raw)
    if _NC_CACHE is None:
        _NC_CACHE = _build_nc()
    nc = _NC_CACHE
    res = bass_utils.run_bass_kernel_spmd(nc, in_maps, core_ids=list(range(N_CORES)))
    out = np.concatenate([res.results[i]["out"] for i in range(N_CORES)], axis=0)
    return out.reshape(BATCH, 1).astype(np.float32)


if __name__ == "__main__":
    x = np.random.randn(BATCH, D).astype(np.float32)
    W = (np.random.randn(1, P_FULL) * 0.02).astype(np.float32)
    b = np.zeros((1,), np.float32)
    M_raw = np.zeros((), np.float32)
    out = kernel(x, W, b, M_raw)
    print("out shape:", out.shape, out.dtype, out[:4, 0])


# revision 7
# speedup vs baseline: 1.8087x; 1.1185x over previous
"""Trainium2 Bass kernel for nn_LogisticRegressionModel (polynomial-feature logistic regression).

Math: the reference computes sigmoid(poly_features(x) @ W.T + b) where poly_features
are all monomials of x (dim 16) up to degree 4, each degree soft-weighted by
w_d = sigmoid(10*(M - d + 0.5)), M = sigmoid(M_raw)*3 + 1.

Every monomial of degree <= 4 over x embeds as a degree-4 monomial over x1 = [x, 1]
(pad with the constant slot, index 16). Folding W, b, M_raw into a symmetrized
coefficient tensor S4 [289, 289] (built on host, O(P) work), the model becomes
logit_i = (x1 (x) x1)^T S4 (x1 (x) x1). The outer product is symmetric, so it is
further folded onto the 153 unordered pairs of 17 symbols using a mod-17 "wrap"
enumeration p=(d,j) <-> {j, (j+d)%17}, d=0..8: S153 = B^T S4 B. The wrap pairs
have regular strides against a doubled x1 buffer, so one DVE tensor_tensor with
broadcast APs builds XXs for FOUR batch tiles at once. Then PE transposes XXs
(2 chunks/tile) into a shared PSUM bank, one batched scalar copy moves 4 tiles of
XXs^T to SBUF, 2 accumulating matmuls per tile against resident S153 give
Y = XXs @ S153 (4 tiles packed into one 2-bank PSUM tile at 256-elem offsets),
one batched DVE multiply forms XXs*Y for 4 tiles, and a tensor_reduce per 8
tiles produces q; sigmoid; store.

The multiply pipeline runs in bf16 (XXs from fp32 x so products are single-
rounded, S in bf16, transposes + matmuls bf16) with fp32 PSUM accumulation:
q is in [-2.5, 2.5] and sigmoid never saturates, so bf16 keeps max rel err
well under 1e-2 (validated numerically). bf16 runs the PE at 1 cycle/row
(vs 4 for fp32's LOW/HIGH double pass).

Sharding: pure data-parallel over the batch, 4096 rows per core x 8 cores.
"""
import sys
import numpy as np
import ml_dtypes
from itertools import combinations_with_replacement, permutations

sys.path.insert(0, "/opt/trn_rl_repo")

import concourse.bass as bass
import concourse.bacc as bacc
import concourse.tile as tile
from concourse import mybir, masks
from concourse import bass_utils

BATCH = 32768
D = 16
DA = 17            # features + constant slot
ND = 9             # wrap distances 0..8
PD = ND * DA       # 153 unordered pairs
MAX_DEGREE = 4
N_CORES = 8
B_CORE = BATCH // N_CORES   # 4096
N_TILES = B_CORE // 128     # 32
KCH = [128, PD - 128]       # 153 split across partition chunks
G = 4                       # tiles per batch group
NG = N_TILES // G           # 8 groups
NXCH = 4                    # x DMA chunks (2 groups each), each contiguous in DRAM
T_PER_CH = N_TILES // NXCH
P_FULL = 1 + sum(
    len(list(combinations_with_replacement(range(D), d))) for d in range(1, MAX_DEGREE + 1)
)
BF16 = ml_dtypes.bfloat16


def _build_s153(W, b, M_raw):
    """Fold W, b and the soft degree weights into the symmetric quartic
    coefficient matrix over the 153 wrap-encoded unordered pairs."""
    W = np.asarray(W, np.float64)
    bval = float(np.asarray(b).reshape(-1)[0])
    M = 1.0 / (1.0 + np.exp(-float(np.asarray(M_raw)))) * (MAX_DEGREE - 1) + 1.0
    coef = {(16, 16, 16, 16): float(W[0, 0]) + bval}
    col = 1
    for d in range(1, MAX_DEGREE + 1):
        w_d = 1.0 / (1.0 + np.exp(-10.0 * (M - d + 0.5)))
        for t in combinations_with_replacement(range(D), d):
            tup = tuple(sorted(t + (16,) * (4 - d)))
            coef[tup] = float(W[0, col]) * w_d
            col += 1
    assert col == P_FULL
    S4 = np.zeros((DA * DA, DA * DA), np.float64)
    for tup, c in coef.items():
        perms = set(permutations(tup))
        v = c / len(perms)
        for (a, b2, c2, d2) in perms:
            S4[a * DA + b2, c2 * DA + d2] += v
    # fold ordered 289-space onto wrap-encoded 153-space
    lookup = {}
    for p, (a, c) in enumerate((j, (j + dd) % DA) for dd in range(ND) for j in range(DA)):
        lookup[(a, c)] = p
        lookup[(c, a)] = p
    B = np.zeros((DA * DA, PD))
    for j in range(DA):
        for k in range(DA):
            B[j * DA + k, lookup[(j, k)]] = 1.0
    return (B.T @ S4 @ B).astype(np.float32)


def _build_nc():
    nc = bacc.Bacc("TRN2", target_bir_lowering=False, debug=False, enable_asserts=False)
    # host pre-packs x1 per core as NXCH contiguous chunks [128, T_PER_CH*34] fp32
    # (tile t of chunk k holds x1 rows of batch tile k*T_PER_CH+t, doubled for
    # the wrap-window reads)
    x_d = nc.dram_tensor(
        "x", [NXCH * 128, T_PER_CH * 2 * DA], mybir.dt.float32, kind="ExternalInput"
    ).ap()
    s_d = nc.dram_tensor("s4", [PD, PD], mybir.dt.bfloat16, kind="ExternalInput").ap()
    out_d = nc.dram_tensor("out", [B_CORE, 1], mybir.dt.float32, kind="ExternalOutput").ap()

    f32 = mybir.dt.float32
    bf16 = mybir.dt.bfloat16
    with tile.TileContext(nc) as tc:
        with (
            tc.tile_pool(name="const", bufs=1) as const_pool,
            tc.tile_pool(name="xx", bufs=3) as xx_pool,
            tc.tile_pool(name="xxt", bufs=3) as xxt_pool,
            tc.tile_pool(name="prod", bufs=2) as prod_pool,
            tc.tile_pool(name="tr_ps", bufs=2, space="PSUM") as trps_pool,
            tc.tile_pool(name="y_ps", bufs=2, space="PSUM") as yps_pool,
            tc.tile_pool(name="o_ps", bufs=1, space="PSUM") as ops_pool,
        ):
            ident_b = const_pool.tile([128, 128], bf16)
            masks.make_identity(nc, ident_b[:])
            ident_f = const_pool.tile([128, 128], f32)
            masks.make_identity(nc, ident_f[:])
            # prewarm the ACT tables (copy + sigmoid) while DMAs are in flight
            warm = const_pool.tile([1, 2], f32)
            nc.scalar.copy(out=warm[:, 0:1], in_=ident_f[0:1, 0:1])
            nc.scalar.activation(
                warm[:, 1:2], warm[:, 0:1], mybir.ActivationFunctionType.Sigmoid
            )
            # S153 chunks: chunk c lives at [:KCH[c], c*153:(c+1)*153]
            s_sb = const_pool.tile([128, 2 * PD], bf16)
            for c in range(2):
                nc.sync.dma_start(
                    out=s_sb[: KCH[c], c * PD : (c + 1) * PD],
                    in_=s_d[c * 128 : c * 128 + KCH[c], :],
                )
            qall = const_pool.tile([128, N_TILES], f32)
            oall = const_pool.tile([128, N_TILES], f32)
            # x arrives in NXCH contiguous chunks so compute starts after the first
            xch = []
            for k in range(NXCH):
                xc = const_pool.tile([128, T_PER_CH * 2 * DA], f32)
                nc.sync.dma_start(out=xc[:], in_=x_d[k * 128 : (k + 1) * 128, :])
                xch.append(xc)

            prodall = None
            for g in range(NG):
                # XXs[p, t4, d*17+j] = x1[p,j]*x1[p,(j+d)%17] for 4 tiles — one op
                xx4 = xx_pool.tile([128, G * PD], bf16)
                ch = xch[g // (NG // NXCH)]
                t_in_ch = (g % (NG // NXCH)) * G
                xc4 = ch[:, t_in_ch * 2 * DA : (t_in_ch + G) * 2 * DA]
                part = list(xc4.ap[0])
                in0 = bass.AP(
                    xc4.tensor, xc4.offset, [part, [2 * DA, G], [0, ND], [1, DA]]
                )
                in1 = bass.AP(
                    xc4.tensor, xc4.offset, [part, [2 * DA, G], [1, ND], [1, DA]]
                )
                eng = nc.vector if g % 2 == 0 else nc.gpsimd
                eng.tensor_tensor(
                    out=xx4[:].rearrange("p (t4 d j) -> p t4 d j", t4=G, d=ND),
                    in0=in0,
                    in1=in1,
                    op=mybir.AluOpType.mult,
                )

                # PE transposes: per tile 2 chunks into one shared PSUM bank
                trp = trps_pool.tile([128, G * 256], bf16)
                for t4 in range(G):
                    xx = xx4[:, t4 * PD : (t4 + 1) * PD]
                    for c in range(2):
                        nc.tensor.transpose(
                            out=trp[: KCH[c], t4 * 256 + c * 128 : t4 * 256 + c * 128 + 128],
                            in_=xx[:, c * 128 : c * 128 + KCH[c]],
                            identity=ident_b[:],
                        )
                # one batched PSUM->SBUF copy for the whole group
                xxt = xxt_pool.tile([128, G * 256], bf16)
                nc.scalar.copy(out=xxt[:], in_=trp[:])

                # Y = XXs @ S153 per tile, packed at 256-elem offsets in a 2-bank
                # PSUM tile (each 153-wide slice stays inside one 2KB bank)
                y4 = yps_pool.tile([128, G * 256], f32)
                for t4 in range(G):
                    for c in range(2):
                        nc.tensor.matmul(
                            out=y4[:, t4 * 256 : t4 * 256 + PD],
                            lhsT=xxt[: KCH[c], t4 * 256 + c * 128 : t4 * 256 + c * 128 + 128],
                            rhs=s_sb[: KCH[c], c * PD : (c + 1) * PD],
                            start=(c == 0),
                            stop=(c == 1),
                        )

                # prod = XXs * Y for 4 tiles in one DVE op
                if g % 2 == 0:
                    prodall = prod_pool.tile([128, 2 * G * PD], bf16)
                y4_ap = bass.AP(y4[:].tensor, y4[:].offset, [list(y4[:].ap[0]), [256, G], [1, PD]])
                nc.vector.tensor_tensor(
                    out=prodall[:, (g % 2) * G * PD : (g % 2 + 1) * G * PD].rearrange(
                        "p (t4 r) -> p t4 r", t4=G
                    ),
                    in0=xx4[:].rearrange("p (t4 r) -> p t4 r", t4=G),
                    in1=y4_ap,
                    op=mybir.AluOpType.mult,
                )
                # q = rowsum over pairs, 8 tiles at a time
                if g % 2 == 1:
                    nc.vector.tensor_reduce(
                        out=qall[:, (g - 1) * G : (g + 1) * G],
                        in_=prodall[:].rearrange("p (t8 r) -> p t8 r", t8=2 * G),
                        axis=mybir.AxisListType.X,
                        op=mybir.AluOpType.add,
                    )

            # sigmoid over all 32 tile-columns at once
            nc.scalar.activation(oall[:], qall[:], mybir.ActivationFunctionType.Sigmoid)
            # transpose [128, 32] -> [32, 128] so the DRAM store is contiguous
            o_ps = ops_pool.tile([N_TILES, 128], f32)
            nc.tensor.transpose(out=o_ps[:], in_=oall[:], identity=ident_f[:])
            o_sb = const_pool.tile([N_TILES, 128], f32)
            nc.vector.tensor_copy(out=o_sb[:], in_=o_ps[:])
            nc.sync.dma_start(
                out=out_d.rearrange("(t p) one -> t (p one)", p=128),
                in_=o_sb[:],
            )
    nc.compile()
    return nc


_NC_CACHE = None


def _pack_inputs(x, W, b, M_raw):
    x = np.asarray(x, np.float32)
    x1 = np.concatenate([x, np.ones((x.shape[0], 1), np.float32)], axis=1)
    # pack per core: [T, 128, 17] -> [NXCH, 128, T_PER_CH, 34] contiguous chunks
    xr = x1.reshape(N_CORES, NXCH, T_PER_CH, 128, DA)
    xp = np.concatenate([xr, xr], axis=4).transpose(0, 1, 3, 2, 4)
    xp = np.ascontiguousarray(
        xp.reshape(N_CORES, NXCH * 128, T_PER_CH * 2 * DA)
    )
    S = _build_s153(W, b, M_raw).astype(BF16)
    return [{"x": xp[i], "s4": S} for i in range(N_CORES)]


def kernel(x, W, b, M_raw):
    global _NC_CACHE
    in_maps = _pack_inputs(x, W, b, M_raw)
    if _NC_CACHE is None:
        _NC_CACHE = _build_nc()
    nc = _NC_CACHE
    res = bass_utils.run_bass_kernel_spmd(nc, in_maps, core_ids=list(range(N_CORES)))
    out = np.concatenate([res.results[i]["out"] for i in range(N_CORES)], axis=0)
    return out.reshape(BATCH, 1).astype(np.float32)


if __name__ == "__main__":
    x = np.random.randn(BATCH, D).astype(np.float32)
    W = (np.random.randn(1, P_FULL) * 0.02).astype(np.float32)
    b = np.zeros((1,), np.float32)
    M_raw = np.zeros((), np.float32)
    out = kernel(x, W, b, M_raw)
    print("out shape:", out.shape, out.dtype, out[:4, 0])


# revision 9
# speedup vs baseline: 1.8205x; 1.0065x over previous
"""Trainium2 Bass kernel for nn_LogisticRegressionModel (polynomial-feature logistic regression).

Math: the reference computes sigmoid(poly_features(x) @ W.T + b) where poly_features
are all monomials of x (dim 16) up to degree 4, each degree soft-weighted by
w_d = sigmoid(10*(M - d + 0.5)), M = sigmoid(M_raw)*3 + 1.

Every monomial of degree <= 4 over x embeds as a degree-4 monomial over x1 = [x, 1]
(pad with the constant slot, index 16). Folding W, b, M_raw into a symmetrized
coefficient tensor S4 [289, 289] (built on host, O(P) work), the model becomes
logit_i = (x1 (x) x1)^T S4 (x1 (x) x1). The outer product is symmetric, so it is
further folded onto the 153 unordered pairs of 17 symbols using a mod-17 "wrap"
enumeration p=(d,j) <-> {j, (j+d)%17}, d=0..8: S153 = B^T S4 B. The wrap pairs
have regular strides against a doubled x1 buffer, so one DVE tensor_tensor with
broadcast APs builds XXs for FOUR batch tiles at once. Then PE transposes XXs
(2 chunks/tile) into a shared PSUM bank, one batched scalar copy moves 4 tiles of
XXs^T to SBUF, 2 accumulating matmuls per tile against resident S153 give
Y = XXs @ S153 (4 tiles packed into one 2-bank PSUM tile at 256-elem offsets),
one batched DVE multiply forms XXs*Y for 4 tiles, and a tensor_reduce per 8
tiles produces q; sigmoid; store.

The multiply pipeline runs in bf16 (XXs from fp32 x so products are single-
rounded, S in bf16, transposes + matmuls bf16) with fp32 PSUM accumulation:
q is in [-2.5, 2.5] and sigmoid never saturates, so bf16 keeps max rel err
well under 1e-2 (validated numerically). bf16 runs the PE at 1 cycle/row
(vs 4 for fp32's LOW/HIGH double pass).

Sharding: pure data-parallel over the batch, 4096 rows per core x 8 cores.
"""
import sys
import numpy as np
import ml_dtypes
from itertools import combinations_with_replacement, permutations

sys.path.insert(0, "/opt/trn_rl_repo")

import concourse.bass as bass
import concourse.bacc as bacc
import concourse.tile as tile
from concourse import mybir, masks
from concourse import bass_utils

BATCH = 32768
D = 16
DA = 17            # features + constant slot
ND = 9             # wrap distances 0..8
PD = ND * DA       # 153 unordered pairs
MAX_DEGREE = 4
N_CORES = 8
B_CORE = BATCH // N_CORES   # 4096
N_TILES = B_CORE // 128     # 32
KCH = [128, PD - 128]       # 153 split across partition chunks
G = 4                       # tiles per batch group
NG = N_TILES // G           # 8 groups
NXCH = 4                    # x DMA chunks (2 groups each), each contiguous in DRAM
T_PER_CH = N_TILES // NXCH
P_FULL = 1 + sum(
    len(list(combinations_with_replacement(range(D), d))) for d in range(1, MAX_DEGREE + 1)
)
BF16 = ml_dtypes.bfloat16


def _build_s153(W, b, M_raw):
    """Fold W, b and the soft degree weights into the symmetric quartic
    coefficient matrix over the 153 wrap-encoded unordered pairs."""
    W = np.asarray(W, np.float64)
    bval = float(np.asarray(b).reshape(-1)[0])
    M = 1.0 / (1.0 + np.exp(-float(np.asarray(M_raw)))) * (MAX_DEGREE - 1) + 1.0
    coef = {(16, 16, 16, 16): float(W[0, 0]) + bval}
    col = 1
    for d in range(1, MAX_DEGREE + 1):
        w_d = 1.0 / (1.0 + np.exp(-10.0 * (M - d + 0.5)))
        for t in combinations_with_replacement(range(D), d):
            tup = tuple(sorted(t + (16,) * (4 - d)))
            coef[tup] = float(W[0, col]) * w_d
            col += 1
    assert col == P_FULL
    S4 = np.zeros((DA * DA, DA * DA), np.float64)
    for tup, c in coef.items():
        perms = set(permutations(tup))
        v = c / len(perms)
        for (a, b2, c2, d2) in perms:
            S4[a * DA + b2, c2 * DA + d2] += v
    # fold ordered 289-space onto wrap-encoded 153-space
    lookup = {}
    for p, (a, c) in enumerate((j, (j + dd) % DA) for dd in range(ND) for j in range(DA)):
        lookup[(a, c)] = p
        lookup[(c, a)] = p
    B = np.zeros((DA * DA, PD))
    for j in range(DA):
        for k in range(DA):
            B[j * DA + k, lookup[(j, k)]] = 1.0
    return (B.T @ S4 @ B).astype(np.float32)


def _build_nc():
    nc = bacc.Bacc("TRN2", target_bir_lowering=False, debug=False, enable_asserts=False)
    # host pre-packs x1 per core as NXCH contiguous chunks [128, T_PER_CH*34] fp32
    # (tile t of chunk k holds x1 rows of batch tile k*T_PER_CH+t, doubled for
    # the wrap-window reads)
    x_d = nc.dram_tensor(
        "x", [NXCH * 128, T_PER_CH * 2 * DA], mybir.dt.float32, kind="ExternalInput"
    ).ap()
    s_d = nc.dram_tensor("s4", [PD, PD], mybir.dt.bfloat16, kind="ExternalInput").ap()
    out_d = nc.dram_tensor("out", [B_CORE, 1], mybir.dt.float32, kind="ExternalOutput").ap()

    f32 = mybir.dt.float32
    bf16 = mybir.dt.bfloat16
    with tile.TileContext(nc) as tc:
        with (
            tc.tile_pool(name="const", bufs=1) as const_pool,
            tc.tile_pool(name="xx", bufs=3) as xx_pool,
            tc.tile_pool(name="xxt", bufs=3) as xxt_pool,
            tc.tile_pool(name="prod", bufs=2) as prod_pool,
            tc.tile_pool(name="tr_ps", bufs=2, space="PSUM") as trps_pool,
            tc.tile_pool(name="y_ps", bufs=2, space="PSUM") as yps_pool,
            tc.tile_pool(name="o_ps", bufs=1, space="PSUM") as ops_pool,
        ):
            ident_b = const_pool.tile([128, 128], bf16)
            masks.make_identity(nc, ident_b[:])
            ident_f = const_pool.tile([128, 128], f32)
            masks.make_identity(nc, ident_f[:])
            # prewarm the ACT tables (copy + sigmoid) while DMAs are in flight
            warm = const_pool.tile([1, 2], f32)
            nc.scalar.copy(out=warm[:, 0:1], in_=ident_f[0:1, 0:1])
            nc.scalar.activation(
                warm[:, 1:2], warm[:, 0:1], mybir.ActivationFunctionType.Sigmoid
            )
            # S153 chunks: chunk c lives at [:KCH[c], c*153:(c+1)*153]
            s_sb = const_pool.tile([128, 2 * PD], bf16)
            for c in range(2):
                nc.sync.dma_start(
                    out=s_sb[: KCH[c], c * PD : (c + 1) * PD],
                    in_=s_d[c * 128 : c * 128 + KCH[c], :],
                )
            qall = const_pool.tile([128, N_TILES], f32)
            oall = const_pool.tile([128, N_TILES], f32)
            # x arrives in NXCH contiguous chunks so compute starts after the first
            xch = []
            for k in range(NXCH):
                xc = const_pool.tile([128, T_PER_CH * 2 * DA], f32)
                nc.sync.dma_start(out=xc[:], in_=x_d[k * 128 : (k + 1) * 128, :])
                xch.append(xc)

            # Warm up the tensor engine while the DMAs land: HAM throttling keeps
            # the PE at 1.2 GHz until it has run ~3.4us continuously; burn that
            # window on dummy transposes so real matmuls run at 2.4 GHz.
            trash = ops_pool.tile([128, 128], bf16)
            for _ in range(14):
                nc.tensor.transpose(out=trash[:], in_=ident_b[:], identity=ident_b[:])

            prodall = None
            for g in range(NG):
                # XXs[p, t4, d*17+j] = x1[p,j]*x1[p,(j+d)%17] for 4 tiles — one op
                xx4 = xx_pool.tile([128, G * PD], bf16)
                ch = xch[g // (NG // NXCH)]
                t_in_ch = (g % (NG // NXCH)) * G
                xc4 = ch[:, t_in_ch * 2 * DA : (t_in_ch + G) * 2 * DA]
                part = list(xc4.ap[0])
                in0 = bass.AP(
                    xc4.tensor, xc4.offset, [part, [2 * DA, G], [0, ND], [1, DA]]
                )
                in1 = bass.AP(
                    xc4.tensor, xc4.offset, [part, [2 * DA, G], [1, ND], [1, DA]]
                )
                eng = nc.vector if g % 8 < 3 else nc.gpsimd
                eng.tensor_tensor(
                    out=xx4[:].rearrange("p (t4 d j) -> p t4 d j", t4=G, d=ND),
                    in0=in0,
                    in1=in1,
                    op=mybir.AluOpType.mult,
                )

                # PE transposes: per tile 2 chunks into one shared PSUM bank
                trp = trps_pool.tile([128, G * 256], bf16)
                for t4 in range(G):
                    xx = xx4[:, t4 * PD : (t4 + 1) * PD]
                    for c in range(2):
                        nc.tensor.transpose(
                            out=trp[: KCH[c], t4 * 256 + c * 128 : t4 * 256 + c * 128 + 128],
                            in_=xx[:, c * 128 : c * 128 + KCH[c]],
                            identity=ident_b[:],
                        )
                # one batched PSUM->SBUF copy for the whole group
                xxt = xxt_pool.tile([128, G * 256], bf16)
                nc.scalar.copy(out=xxt[:], in_=trp[:])

                # Y = XXs @ S153 per tile, packed at 256-elem offsets in a 2-bank
                # PSUM tile (each 153-wide slice stays inside one 2KB bank)
                y4 = yps_pool.tile([128, G * 256], f32)
                for t4 in range(G):
                    for c in range(2):
                        nc.tensor.matmul(
                            out=y4[:, t4 * 256 : t4 * 256 + PD],
                            lhsT=xxt[: KCH[c], t4 * 256 + c * 128 : t4 * 256 + c * 128 + 128],
                            rhs=s_sb[: KCH[c], c * PD : (c + 1) * PD],
                            start=(c == 0),
                            stop=(c == 1),
                        )

                # prod = XXs * Y for 4 tiles in one DVE op
                if g % 2 == 0:
                    prodall = prod_pool.tile([128, 2 * G * PD], bf16)
                y4_ap = bass.AP(y4[:].tensor, y4[:].offset, [list(y4[:].ap[0]), [256, G], [1, PD]])
                nc.vector.tensor_tensor(
                    out=prodall[:, (g % 2) * G * PD : (g % 2 + 1) * G * PD].rearrange(
                        "p (t4 r) -> p t4 r", t4=G
                    ),
                    in0=xx4[:].rearrange("p (t4 r) -> p t4 r", t4=G),
                    in1=y4_ap,
                    op=mybir.AluOpType.mult,
                )
                # q = rowsum over pairs, 8 tiles at a time
                if g % 2 == 1:
                    nc.vector.tensor_reduce(
                        out=qall[:, (g - 1) * G : (g + 1) * G],
                        in_=prodall[:].rearrange("p (t8 r) -> p t8 r", t8=2 * G),
                        axis=mybir.AxisListType.X,
                        op=mybir.AluOpType.add,
                    )

            # sigmoid over all 32 tile-columns at once
            nc.scalar.activation(oall[:], qall[:], mybir.ActivationFunctionType.Sigmoid)
            # transpose [128, 32] -> [32, 128] so the DRAM store is contiguous
            o_ps = ops_pool.tile([N_TILES, 128], f32)
            nc.tensor.transpose(out=o_ps[:], in_=oall[:], identity=ident_f[:])
            o_sb = const_pool.tile([N_TILES, 128], f32)
            nc.vector.tensor_copy(out=o_sb[:], in_=o_ps[:])
            nc.sync.dma_start(
                out=out_d.rearrange("(t p) one -> t (p one)", p=128),
                in_=o_sb[:],
            )
    nc.compile()
    return nc


_NC_CACHE = None


def _pack_inputs(x, W, b, M_raw):
    x = np.asarray(x, np.float32)
    x1 = np.concatenate([x, np.ones((x.shape[0], 1), np.float32)], axis=1)
    # pack per core: [T, 128, 17] -> [NXCH, 128, T_PER_CH, 34] contiguous chunks
    xr = x1.reshape(N_CORES, NXCH, T_PER_CH, 128, DA)
    xp = np.concatenate([xr, xr], axis=4).transpose(0, 1, 3, 2, 4)
    xp = np.ascontiguousarray(
        xp.reshape(N_CORES, NXCH * 128, T_PER_CH * 2 * DA)
    )
    S = _build_s153(W, b, M_raw).astype(BF16)
    return [{"x": xp[i], "s4": S} for i in range(N_CORES)]


def kernel(x, W, b, M_raw):
    global _NC_CACHE
    in_maps = _pack_inputs(x, W, b, M_raw)
    if _NC_CACHE is None:
        _NC_CACHE = _build_nc()
    nc = _NC_CACHE
    res = bass_utils.run_bass_kernel_spmd(nc, in_maps, core_ids=list(range(N_CORES)))
    out = np.concatenate([res.results[i]["out"] for i in range(N_CORES)], axis=0)
    return out.reshape(BATCH, 1).astype(np.float32)


if __name__ == "__main__":
    x = np.random.randn(BATCH, D).astype(np.float32)
    W = (np.random.randn(1, P_FULL) * 0.02).astype(np.float32)
    b = np.zeros((1,), np.float32)
    M_raw = np.zeros((), np.float32)
    out = kernel(x, W, b, M_raw)
    print("out shape:", out.shape, out.dtype, out[:4, 0])
